# revision 12
# baseline (speedup 1.0000x reference)
"""Trainium2 Bass kernel for CausalSelfAttention with adapter prompt.

Strategy (8 NeuronCores, tensor-parallel over heads):
  - Each core owns 4 of the 32 heads: it gets the matching column slices of
    w_attn (q/k/v), computes qkv for all tokens, runs causal attention +
    adapter attention for its heads, producing its 512-row slice of y^T.
  - AllGather of y^T slices -> full y^T [4096, 4096] on every core.
  - Each core computes a 512-column slice of y @ w_proj; the host
    concatenates the 8 column slices.

Layout choices:
  - x is pre-transposed on host to xT [C, B*T] so qT/kT come out of the PE
    in [head_dim, token] layout and v in [token, head_dim] layout.
  - Within each head, q/k dims are permuted to [evens, odds] so RoPE becomes
    four [64, n] elementwise multiplies with host-precomputed cos/sin tables.
    q and k use the same permutation, so dot products are unchanged.
  - Attention works on S^T [tk, tq] tiles: S^T = (kT chunk).T @ qT, exp on
    the scalar engine (scores are bounded, no max subtraction needed),
    causal masking via block skipping + precomputed triangular masks, PV
    accumulated in PSUM with v chunks as the stationary operand, and the
    softmax denominator via an all-ones stationary matmul (result is
    broadcast across partitions for the final divide).
  - All matmul inputs are float32r (full PE rate at moving dim 512).
"""

import numpy as np

import concourse.bass as bass
import concourse.mybir as mybir
from concourse import bacc
from concourse.tile import TileContext
from concourse.bass_utils import run_bass_kernel_spmd

F32 = mybir.dt.float32
F32R = mybir.dt.float32r
EXP = mybir.ActivationFunctionType.Exp

NC = 8  # cores
B, T, C = 2, 2048, 4096
NT = B * T  # 4096 tokens
H = 32
HS = 128  # head size
HPC = H // NC  # 4 heads per core
F = HPC * HS  # 512 features per core per q/k/v
AT = 10  # adapter tokens
CCH = C // 128  # 32 contraction chunks
SCALE = 1.0 / float(np.sqrt(HS))
ASCALE = 1.0 / float(np.sqrt(C))

_NC_CACHE = {}


def _build():
    if "nc" in _NC_CACHE:
        return _NC_CACHE["nc"]
    nc = bacc.Bacc("TRN2", target_bir_lowering=False, debug=False, num_devices=NC)

    # ---- external I/O (per core) ----
    xt = nc.dram_tensor("xt", [C, NT], F32, kind="ExternalInput")
    wqk = nc.dram_tensor("wqk", [C, 2 * F], F32, kind="ExternalInput")
    wv = nc.dram_tensor("wv", [C, F], F32, kind="ExternalInput")
    wp = nc.dram_tensor("wp", [C, F], F32, kind="ExternalInput")
    cost = nc.dram_tensor("cost", [128, T], F32, kind="ExternalInput")
    sint = nc.dram_tensor("sint", [128, T], F32, kind="ExternalInput")
    masks = nc.dram_tensor("masks", [512, 512], F32, kind="ExternalInput")
    onesd = nc.dram_tensor("onesd", [128, 128], F32, kind="ExternalInput")
    gbd = nc.dram_tensor("gbd", [128, 1], F32, kind="ExternalInput")
    apt = nc.dram_tensor("apt", [C, AT], F32, kind="ExternalInput")
    out = nc.dram_tensor("out", [NT, F], F32, kind="ExternalOutput")

    # ---- internal DRAM ----
    qkt_d = nc.dram_tensor("qkt_d", [2 * F, NT], F32R, kind="Internal")
    v_d = nc.dram_tensor("v_d", [NT, F], F32R, kind="Internal")
    yt_in = nc.dram_tensor("yt_in", [F, NT], F32, kind="Internal")
    yt_all = nc.dram_tensor(
        "yt_all", [C, NT], F32, kind="Internal", addr_space="Shared"
    )

    with TileContext(nc) as tc:
        with tc.tile_pool(name="const", bufs=1) as cpool:
            cos_t = cpool.tile([128, T], F32)
            sin_t = cpool.tile([128, T], F32)
            nc.sync.dma_start(cos_t[:], cost[:])
            nc.sync.dma_start(sin_t[:], sint[:])
            ones_t = cpool.tile([128, 128], F32R)
            nc.sync.dma_start(ones_t[:], onesd[:].bitcast(F32R))
            gb_t = cpool.tile([128, 1], F32)
            nc.sync.dma_start(gb_t[:], gbd[:])
            mask_ts = []
            for r in range(4):
                mt = cpool.tile([128, 512], F32, name=f"mask{r}")
                nc.sync.dma_start(mt[:], masks[128 * r : 128 * (r + 1), :])
                mask_ts.append(mt)
            akt_sb = cpool.tile([128, HPC * AT], F32R)  # k-adapter, per head cols
            av_sb = cpool.tile([AT, F], F32R)

            # ================= Phase 1: qkv projections =================
            # three passes (q, k, v); each keeps its 8MB weight slice
            # resident and streams xT once.
            for piece in range(3):  # 0=q, 1=k, 2=v
                with tc.tile_pool(name=f"w{piece}", bufs=1) as wpool, tc.tile_pool(
                    name=f"xs{piece}", bufs=6
                ) as xpool, tc.tile_pool(name=f"ev{piece}", bufs=4) as epool, tc.tile_pool(
                    name=f"ps{piece}", bufs=2, space="PSUM"
                ) as pspool:
                    w_ts = []
                    for cc in range(CCH):
                        wt = wpool.tile([128, F], F32R, name=f"w{piece}_{cc}")
                        src = wqk if piece < 2 else wv
                        off = piece * F if piece < 2 else 0
                        nc.sync.dma_start(
                            wt[:], src[128 * cc : 128 * (cc + 1), off : off + F].bitcast(F32R)
                        )
                        w_ts.append(wt)
                    for m in range(NT // 512):
                        ps = [
                            pspool.tile([128, 512], F32, name=f"ps{piece}_{m}_{f}", tag=f"ps{f}")
                            for f in range(4)
                        ]
                        for cc in range(CCH):
                            xt_t = xpool.tile([128, 512], F32R, name=f"x{piece}_{m}_{cc}", tag="x")
                            nc.sync.dma_start(
                                xt_t[:],
                                xt[128 * cc : 128 * (cc + 1), 512 * m : 512 * (m + 1)].bitcast(F32R),
                            )
                            st = cc == 0
                            sp = cc == CCH - 1
                            if piece < 2:
                                for f in range(4):
                                    nc.tensor.matmul(
                                        ps[f][:],
                                        w_ts[cc][:, 128 * f : 128 * (f + 1)],
                                        xt_t[:],
                                        start=st,
                                        stop=sp,
                                    )
                            else:
                                for tb in range(4):
                                    nc.tensor.matmul(
                                        ps[tb][:],
                                        xt_t[:, 128 * tb : 128 * (tb + 1)],
                                        w_ts[cc][:],
                                        start=st,
                                        stop=sp,
                                    )
                        tloc = 512 * (m % (T // 512))  # token pos within batch
                        csl = cos_t[:, tloc : tloc + 512]
                        ssl = sin_t[:, tloc : tloc + 512]
                        if piece < 2:
                            # rope with head-pair layout: chunk 2g holds the
                            # even dims of heads (2g, 2g+1); chunk 2g+1 the
                            # odd dims.  cos/sin tables are duplicated across
                            # both partition halves, so every op is a full
                            # 128-partition, base-aligned tensor op.
                            for g in range(2):
                                psA = ps[2 * g][:]
                                psB = ps[2 * g + 1][:]
                                s1 = epool.tile([128, 512], F32, name=f"s1_{piece}_{m}_{g}", tag="s1")
                                s2 = epool.tile([128, 512], F32, name=f"s2_{piece}_{m}_{g}", tag="s2")
                                s3 = epool.tile([128, 512], F32, name=f"s3_{piece}_{m}_{g}", tag="s3")
                                s4 = epool.tile([128, 512], F32, name=f"s4_{piece}_{m}_{g}", tag="s4")
                                rotE = epool.tile([128, 512], F32R, name=f"re_{piece}_{m}_{g}", tag="rotE")
                                rotO = epool.tile([128, 512], F32R, name=f"ro_{piece}_{m}_{g}", tag="rotO")
                                nc.vector.tensor_mul(s1[:], psA, csl)
                                nc.vector.tensor_mul(s2[:], psB, ssl)
                                nc.vector.tensor_sub(rotE[:], s1[:], s2[:])
                                nc.vector.tensor_mul(s3[:], psA, ssl)
                                nc.vector.tensor_mul(s4[:], psB, csl)
                                nc.vector.tensor_add(rotO[:], s3[:], s4[:])
                                base = piece * F + 256 * g
                                nc.sync.dma_start(
                                    qkt_d[base : base + 128, 512 * m : 512 * (m + 1)],
                                    rotE[:],
                                )
                                nc.sync.dma_start(
                                    qkt_d[base + 128 : base + 256, 512 * m : 512 * (m + 1)],
                                    rotO[:],
                                )
                        else:
                            for f in range(4):
                                vsb = epool.tile([128, F], F32R, name=f"vsb_{m}_{f}", tag="vsb")
                                nc.vector.tensor_copy(vsb[:], ps[f][:])
                                nc.sync.dma_start(
                                    v_d[512 * m + 128 * f : 512 * m + 128 * (f + 1), :],
                                    vsb[:],
                                )
                    # adapter matmuls while this piece's weights are resident
                    if piece == 1:
                        # adapter k: per head, gather its even (chunk 2g) and
                        # odd (chunk 2g+1) dims into one [128, AT] column
                        # block so akt_sb[:, AT*h:+AT] is a ready lhsT.
                        with tc.tile_pool(name="apk", bufs=1) as apool:
                            akpsE = pspool.tile([64, HPC * AT], F32, name="akpsE", tag="ps0")
                            akpsO = pspool.tile([64, HPC * AT], F32, name="akpsO", tag="ps1")
                            for cc in range(CCH):
                                ap_t = apool.tile([128, AT], F32R, name=f"apk_{cc}", tag="ap")
                                nc.sync.dma_start(
                                    ap_t[:], apt[128 * cc : 128 * (cc + 1), :].bitcast(F32R)
                                )
                                for h in range(HPC):
                                    g, idx = h // 2, h % 2
                                    ecol = 256 * g + 64 * idx
                                    ocol = 256 * g + 128 + 64 * idx
                                    nc.tensor.matmul(
                                        akpsE[:, AT * h : AT * (h + 1)],
                                        w_ts[cc][:, ecol : ecol + 64],
                                        ap_t[:],
                                        start=(cc == 0),
                                        stop=(cc == CCH - 1),
                                    )
                                    nc.tensor.matmul(
                                        akpsO[:, AT * h : AT * (h + 1)],
                                        w_ts[cc][:, ocol : ocol + 64],
                                        ap_t[:],
                                        start=(cc == 0),
                                        stop=(cc == CCH - 1),
                                    )
                            tmpE = apool.tile([64, HPC * AT], F32R, name="tmpE")
                            tmpO = apool.tile([64, HPC * AT], F32R, name="tmpO")
                            nc.vector.tensor_copy(tmpE[:], akpsE[:])
                            nc.vector.tensor_copy(tmpO[:], akpsO[:])
                            nc.sync.dma_start(akt_sb[0:64, :], tmpE[:])
                            nc.sync.dma_start(akt_sb[64:128, :], tmpO[:])
                    if piece == 2:
                        with tc.tile_pool(name="apv", bufs=1) as apool:
                            avps = pspool.tile([AT, F], F32, name="avps", tag="ps0")
                            for cc in range(CCH):
                                ap_t = apool.tile([128, AT], F32R, name=f"apv_{cc}", tag="ap")
                                nc.sync.dma_start(
                                    ap_t[:], apt[128 * cc : 128 * (cc + 1), :].bitcast(F32R)
                                )
                                nc.tensor.matmul(
                                    avps[:],
                                    ap_t[:],
                                    w_ts[cc][:],
                                    start=(cc == 0),
                                    stop=(cc == CCH - 1),
                                )
                            nc.vector.tensor_copy(av_sb[:], avps[:])

            # ================= Phase 2: attention =================
            with tc.tile_pool(name="qk", bufs=2) as qkpool, tc.tile_pool(
                name="vt", bufs=2
            ) as vpool, tc.tile_pool(name="pp", bufs=3) as ppool, tc.tile_pool(
                name="cmb", bufs=2
            ) as cpool2, tc.tile_pool(
                name="sps", bufs=2, space="PSUM"
            ) as spspool, tc.tile_pool(
                name="acc", bufs=1, space="PSUM"
            ) as accpool:
                for b in range(B):
                    for hh in range(HPC):
                        bh = b * HPC + hh
                        qT = qkpool.tile([128, T], F32R, name=f"qT_{bh}", tag="qT")
                        kT = qkpool.tile([128, T], F32R, name=f"kT_{bh}", tag="kT")
                        g, idx = hh // 2, hh % 2
                        erow = 256 * g + 64 * idx
                        orow = 256 * g + 128 + 64 * idx
                        for dst, off in ((qT, 0), (kT, F)):
                            nc.sync.dma_start(
                                dst[0:64, :],
                                qkt_d[off + erow : off + erow + 64, b * T : (b + 1) * T],
                            )
                            nc.sync.dma_start(
                                dst[64:128, :],
                                qkt_d[off + orow : off + orow + 64, b * T : (b + 1) * T],
                            )
                        v_ts = []
                        for j in range(T // 128):
                            vt_ = vpool.tile([128, 128], F32R, name=f"v_{bh}_{j}", tag=f"v{j}")
                            nc.sync.dma_start(
                                vt_[:],
                                v_d[
                                    b * T + 128 * j : b * T + 128 * (j + 1),
                                    128 * hh : 128 * (hh + 1),
                                ],
                            )
                            v_ts.append(vt_)
                        for i in range(T // 512):
                            nch = 4 * (i + 1)  # causal Tk chunks for this Tq super
                            y_ps = accpool.tile([128, 512], F32, name=f"y_{bh}_{i}", tag="y")
                            d_ps = accpool.tile([128, 512], F32, name=f"d_{bh}_{i}", tag="d")
                            qsl = qT[:, 512 * i : 512 * (i + 1)]
                            for jj in range(nch // 2):
                                j0, j1 = 2 * jj, 2 * jj + 1
                                s_ps = spspool.tile(
                                    [128, 1024], F32, name=f"s_{bh}_{i}_{jj}", tag="s"
                                )
                                nc.tensor.matmul(
                                    s_ps[:, 0:512],
                                    kT[:, 128 * j0 : 128 * (j0 + 1)],
                                    qsl,
                                    start=True,
                                    stop=True,
                                )
                                nc.tensor.matmul(
                                    s_ps[:, 512:1024],
                                    kT[:, 128 * j1 : 128 * (j1 + 1)],
                                    qsl,
                                    start=True,
                                    stop=True,
                                )
                                p_sb = ppool.tile(
                                    [128, 1024], F32R, name=f"p_{bh}_{i}_{jj}", tag="p"
                                )
                                nc.scalar.activation(p_sb[:], s_ps[:], EXP, scale=SCALE)
                                for j in (j0, j1):
                                    r = j - 4 * i
                                    if r >= 0:
                                        sl = p_sb[:, (j - j0) * 512 : (j - j0) * 512 + 512]
                                        nc.vector.tensor_mul(sl, sl, mask_ts[r][:])
                                for j in (j0, j1):
                                    psl = p_sb[:, (j - j0) * 512 : (j - j0) * 512 + 512]
                                    nc.tensor.matmul(
                                        y_ps[:],
                                        v_ts[j][:],
                                        psl,
                                        start=(j == 0),
                                        stop=(j == nch - 1),
                                    )
                                    nc.tensor.matmul(
                                        d_ps[:],
                                        ones_t[:],
                                        psl,
                                        start=(j == 0),
                                        stop=(j == nch - 1),
                                    )
                            # adapter attention for this Tq super
                            as_ps = spspool.tile([AT, 512], F32, name=f"as_{bh}_{i}", tag="s")
                            nc.tensor.matmul(
                                as_ps[:],
                                akt_sb[:, AT * hh : AT * (hh + 1)],
                                qsl,
                                start=True,
                                stop=True,
                            )
                            ap_sb = ppool.tile([AT, 512], F32R, name=f"ap_{bh}_{i}", tag="ap2")
                            nc.scalar.activation(ap_sb[:], as_ps[:], EXP, scale=ASCALE)
                            ya_ps = accpool.tile([128, 512], F32, name=f"ya_{bh}_{i}", tag="ya")
                            da_ps = accpool.tile([128, 512], F32, name=f"da_{bh}_{i}", tag="da")
                            nc.tensor.matmul(
                                ya_ps[:],
                                av_sb[:, 128 * hh : 128 * (hh + 1)],
                                ap_sb[:],
                                start=True,
                                stop=True,
                            )
                            nc.tensor.matmul(
                                da_ps[:], ones_t[0:AT, :], ap_sb[:], start=True, stop=True
                            )
                            # combine: y/denom + g * ya/denom_a
                            rm = cpool2.tile([128, 512], F32, name=f"rm_{bh}_{i}", tag="rm")
                            ra = cpool2.tile([128, 512], F32, name=f"ra_{bh}_{i}", tag="ra")
                            nc.vector.reciprocal(rm[:], d_ps[:])
                            nc.vector.reciprocal(ra[:], da_ps[:])
                            yn = cpool2.tile([128, 512], F32, name=f"yn_{bh}_{i}", tag="yn")
                            tmp = cpool2.tile([128, 512], F32, name=f"tmp_{bh}_{i}", tag="tmp")
                            yout = cpool2.tile([128, 512], F32, name=f"yo_{bh}_{i}", tag="yo")
                            nc.vector.tensor_mul(yn[:], y_ps[:], rm[:])
                            nc.vector.tensor_mul(tmp[:], ya_ps[:], ra[:])
                            nc.vector.scalar_tensor_tensor(
                                yout[:],
                                tmp[:],
                                gb_t[:, 0:1],
                                yn[:],
                                mybir.AluOpType.mult,
                                mybir.AluOpType.add,
                            )
                            nc.sync.dma_start(
                                yt_in[
                                    128 * hh : 128 * (hh + 1),
                                    b * T + 512 * i : b * T + 512 * (i + 1),
                                ],
                                yout[:],
                            )

            # ================= Phase 2.5: AllGather y^T =================
            nc.gpsimd.collective_compute(
                "AllGather",
                mybir.AluOpType.bypass,
                replica_groups=[list(range(NC))],
                ins=[yt_in[:]],
                outs=[yt_all[:]],
            )

            # ================= Phase 3: output projection =================
            with tc.tile_pool(name="wpp", bufs=1) as wpool, tc.tile_pool(
                name="yts", bufs=6
            ) as ypool, tc.tile_pool(name="oev", bufs=4) as opool, tc.tile_pool(
                name="ops", bufs=2, space="PSUM"
            ) as pspool:
                wp_ts = []
                for cc in range(CCH):
                    wt = wpool.tile([128, F], F32R, name=f"wp_{cc}")
                    nc.sync.dma_start(
                        wt[:], wp[128 * cc : 128 * (cc + 1), :].bitcast(F32R)
                    )
                    wp_ts.append(wt)
                for s in range(NT // 512):
                    ps = [
                        pspool.tile([128, 512], F32, name=f"o_{s}_{tb}", tag=f"o{tb}")
                        for tb in range(4)
                    ]
                    for cc in range(CCH):
                        yt_t = ypool.tile([128, 512], F32R, name=f"yt_{s}_{cc}", tag="yt")
                        nc.sync.dma_start(
                            yt_t[:],
                            yt_all[128 * cc : 128 * (cc + 1), 512 * s : 512 * (s + 1)].bitcast(F32R),
                        )
                        for tb in range(4):
                            nc.tensor.matmul(
                                ps[tb][:],
                                yt_t[:, 128 * tb : 128 * (tb + 1)],
                                wp_ts[cc][:],
                                start=(cc == 0),
                                stop=(cc == CCH - 1),
                            )
                    for tb in range(4):
                        osb = opool.tile([128, F], F32, name=f"ob_{s}_{tb}", tag="ob")
                        nc.vector.tensor_copy(osb[:], ps[tb][:])
                        nc.sync.dma_start(
                            out[512 * s + 128 * tb : 512 * s + 128 * (tb + 1), :],
                            osb[:],
                        )
    nc.compile()
    _NC_CACHE["nc"] = nc
    return nc


def _prep_inputs(x, adaption_prompt, rope_cache, w_attn, w_proj, gating_factor):
    x = np.asarray(x, dtype=np.float32)
    w_attn = np.asarray(w_attn, dtype=np.float32)
    w_proj = np.asarray(w_proj, dtype=np.float32)
    rope_cache = np.asarray(rope_cache, dtype=np.float32)
    adaption_prompt = np.asarray(adaption_prompt, dtype=np.float32)
    g = float(np.asarray(gating_factor).reshape(-1)[0])

    xt = np.ascontiguousarray(x.reshape(NT, C).T)
    ang = rope_cache[:T]  # [T, 64]
    cos1 = np.cos(ang).T.astype(np.float32)  # [64, T]
    sin1 = np.sin(ang).T.astype(np.float32)
    cost = np.ascontiguousarray(np.concatenate([cos1, cos1], axis=0))
    sint = np.ascontiguousarray(np.concatenate([sin1, sin1], axis=0))
    ev = np.arange(0, HS, 2)
    od = np.arange(1, HS, 2)
    masks = np.zeros((512, 512), dtype=np.float32)
    tk = np.arange(128)[:, None]
    tq = np.arange(512)[None, :]
    for r in range(4):
        masks[128 * r : 128 * (r + 1)] = (128 * r + tk <= tq).astype(np.float32)
    onesd = np.ones((128, 128), dtype=np.float32)
    gbd = np.full((128, 1), g, dtype=np.float32)
    apt = np.ascontiguousarray(adaption_prompt[0].T)  # [C, AT]

    in_maps = []
    for c in range(NC):
        heads = list(range(HPC * c, HPC * (c + 1)))
        # chunk layout: [h0e|h1e, h0o|h1o, h2e|h3e, h2o|h3o]
        qcols = np.concatenate(
            [
                np.concatenate([heads[2 * g + 0] * HS + par, heads[2 * g + 1] * HS + par])
                for g in range(HPC // 2)
                for par in (ev, od)
            ]
        )
        kcols = C + qcols
        vcols = np.concatenate([2 * C + h * HS + np.arange(HS) for h in heads])
        wqk = np.ascontiguousarray(w_attn[:, np.concatenate([qcols, kcols])])
        wv = np.ascontiguousarray(w_attn[:, vcols])
        wpc = np.ascontiguousarray(w_proj[:, F * c : F * (c + 1)])
        in_maps.append(
            {
                "xt": xt,
                "wqk": wqk,
                "wv": wv,
                "wp": wpc,
                "cost": cost,
                "sint": sint,
                "masks": masks,
                "onesd": onesd,
                "gbd": gbd,
                "apt": apt,
            }
        )
    return in_maps


def kernel(x, adaption_prompt, rope_cache, w_attn, w_proj, gating_factor):
    nc = _build()
    in_maps = _prep_inputs(
        x, adaption_prompt, rope_cache, w_attn, w_proj, gating_factor
    )
    res = run_bass_kernel_spmd(nc, in_maps, core_ids=list(range(NC)))
    full = np.concatenate([res.results[c]["out"] for c in range(NC)], axis=1)
    return full.reshape(B, T, C)


# revision 16
# speedup vs baseline: 1.0365x; 1.0365x over previous
"""Trainium2 Bass kernel for CausalSelfAttention with adapter prompt.

Strategy (8 NeuronCores, tensor-parallel over heads):
  - Each core owns 4 of the 32 heads: it gets the matching column slices of
    w_attn (q/k/v), computes qkv for all tokens, runs causal attention +
    adapter attention for its heads, producing its 512-row slice of y^T.
  - AllGather of y^T slices -> full y^T [4096, 4096] on every core.
  - Each core computes a 512-column slice of y @ w_proj; the host
    concatenates the 8 column slices.

Layout choices:
  - x is pre-transposed on host to xT [C, B*T] so qT/kT come out of the PE
    in [head_dim, token] layout and v in [token, head_dim] layout.
  - Within each head, q/k dims are permuted to [evens, odds] so RoPE becomes
    four [64, n] elementwise multiplies with host-precomputed cos/sin tables.
    q and k use the same permutation, so dot products are unchanged.
  - Attention works on S^T [tk, tq] tiles: S^T = (kT chunk).T @ qT, exp on
    the scalar engine (scores are bounded, no max subtraction needed),
    causal masking via block skipping + precomputed triangular masks, PV
    accumulated in PSUM with v chunks as the stationary operand, and the
    softmax denominator via an all-ones stationary matmul (result is
    broadcast across partitions for the final divide).
  - All matmul inputs are float32r (full PE rate at moving dim 512).
"""

import numpy as np

import concourse.bass as bass
import concourse.mybir as mybir
from concourse import bacc
from concourse.tile import TileContext
from concourse.bass_utils import run_bass_kernel_spmd

F32 = mybir.dt.float32
F32R = mybir.dt.float32r
EXP = mybir.ActivationFunctionType.Exp

NC = 8  # cores
B, T, C = 2, 2048, 4096
NT = B * T  # 4096 tokens
H = 32
HS = 128  # head size
HPC = H // NC  # 4 heads per core
F = HPC * HS  # 512 features per core per q/k/v
AT = 10  # adapter tokens
CCH = C // 128  # 32 contraction chunks
SCALE = 1.0 / float(np.sqrt(HS))
ASCALE = 1.0 / float(np.sqrt(C))

_NC_CACHE = {}


def _build():
    if "nc" in _NC_CACHE:
        return _NC_CACHE["nc"]
    nc = bacc.Bacc("TRN2", target_bir_lowering=False, debug=False, num_devices=NC)

    # ---- external I/O (per core) ----
    xt = nc.dram_tensor("xt", [C, NT], F32, kind="ExternalInput")
    wqk = nc.dram_tensor("wqk", [C, 2 * F], F32, kind="ExternalInput")
    wv = nc.dram_tensor("wv", [C, F], F32, kind="ExternalInput")
    wp = nc.dram_tensor("wp", [C, F], F32, kind="ExternalInput")
    cost = nc.dram_tensor("cost", [128, T], F32, kind="ExternalInput")
    sint = nc.dram_tensor("sint", [128, T], F32, kind="ExternalInput")
    masks = nc.dram_tensor("masks", [512, 512], F32, kind="ExternalInput")
    onesd = nc.dram_tensor("onesd", [128, 128], F32, kind="ExternalInput")
    gbd = nc.dram_tensor("gbd", [128, 1], F32, kind="ExternalInput")
    apt = nc.dram_tensor("apt", [C, AT], F32, kind="ExternalInput")
    out = nc.dram_tensor("out", [NT, F], F32, kind="ExternalOutput")

    # ---- internal DRAM ----
    qkt_d = nc.dram_tensor("qkt_d", [2 * F, NT], F32R, kind="Internal")
    v_d = nc.dram_tensor("v_d", [NT, F], F32R, kind="Internal")
    # y^T staging split per batch so the batch-0 AllGather overlaps batch-1
    # attention and the projection phase.
    yt_in = [
        nc.dram_tensor(f"yt_in{b}", [F, T], F32, kind="Internal") for b in range(B)
    ]
    yt_all = [
        nc.dram_tensor(f"yt_all{b}", [C, T], F32, kind="Internal", addr_space="Shared")
        for b in range(B)
    ]

    with TileContext(nc) as tc:
        with tc.tile_pool(name="const", bufs=1) as cpool:
            cos_t = cpool.tile([128, T], F32)
            sin_t = cpool.tile([128, T], F32)
            nc.sync.dma_start(cos_t[:], cost[:])
            nc.sync.dma_start(sin_t[:], sint[:])
            ones_t = cpool.tile([128, 128], F32R)
            nc.sync.dma_start(ones_t[:], onesd[:].bitcast(F32R))
            gb_t = cpool.tile([128, 1], F32)
            nc.sync.dma_start(gb_t[:], gbd[:])
            mask_ts = []
            for r in range(4):
                mt = cpool.tile([128, 512], F32, name=f"mask{r}")
                nc.sync.dma_start(mt[:], masks[128 * r : 128 * (r + 1), :])
                mask_ts.append(mt)
            akt_sb = cpool.tile([128, HPC * AT], F32R)  # k-adapter, per head cols
            av_sb = cpool.tile([AT, F], F32R)

            # ================= Phase 1: qkv projections =================
            # three passes (q, k, v); each keeps its 8MB weight slice
            # resident and streams xT once.
            for piece in range(3):  # 0=q, 1=k, 2=v
                with tc.tile_pool(name=f"w{piece}", bufs=1) as wpool, tc.tile_pool(
                    name=f"xs{piece}", bufs=6
                ) as xpool, tc.tile_pool(name=f"ev{piece}", bufs=4) as epool, tc.tile_pool(
                    name=f"ps{piece}", bufs=2, space="PSUM"
                ) as pspool:
                    w_ts = []
                    for cc in range(CCH):
                        wt = wpool.tile([128, F], F32R, name=f"w{piece}_{cc}")
                        src = wqk if piece < 2 else wv
                        off = piece * F if piece < 2 else 0
                        nc.sync.dma_start(
                            wt[:], src[128 * cc : 128 * (cc + 1), off : off + F].bitcast(F32R)
                        )
                        w_ts.append(wt)
                    for m in range(NT // 512):
                        ps = [
                            pspool.tile([128, 512], F32, name=f"ps{piece}_{m}_{f}", tag=f"ps{f}")
                            for f in range(4)
                        ]
                        for cc in range(CCH):
                            xt_t = xpool.tile([128, 512], F32R, name=f"x{piece}_{m}_{cc}", tag="x")
                            nc.sync.dma_start(
                                xt_t[:],
                                xt[128 * cc : 128 * (cc + 1), 512 * m : 512 * (m + 1)].bitcast(F32R),
                            )
                            st = cc == 0
                            sp = cc == CCH - 1
                            if piece < 2:
                                for f in range(4):
                                    nc.tensor.matmul(
                                        ps[f][:],
                                        w_ts[cc][:, 128 * f : 128 * (f + 1)],
                                        xt_t[:],
                                        start=st,
                                        stop=sp,
                                    )
                            else:
                                for tb in range(4):
                                    nc.tensor.matmul(
                                        ps[tb][:],
                                        xt_t[:, 128 * tb : 128 * (tb + 1)],
                                        w_ts[cc][:],
                                        start=st,
                                        stop=sp,
                                    )
                        tloc = 512 * (m % (T // 512))  # token pos within batch
                        csl = cos_t[:, tloc : tloc + 512]
                        ssl = sin_t[:, tloc : tloc + 512]
                        if piece < 2:
                            # rope with head-pair layout: chunk 2g holds the
                            # even dims of heads (2g, 2g+1); chunk 2g+1 the
                            # odd dims.  cos/sin tables are duplicated across
                            # both partition halves, so every op is a full
                            # 128-partition, base-aligned tensor op.
                            for g in range(2):
                                psA = ps[2 * g][:]
                                psB = ps[2 * g + 1][:]
                                s1 = epool.tile([128, 512], F32, name=f"s1_{piece}_{m}_{g}", tag="s1")
                                s2 = epool.tile([128, 512], F32, name=f"s2_{piece}_{m}_{g}", tag="s2")
                                s3 = epool.tile([128, 512], F32, name=f"s3_{piece}_{m}_{g}", tag="s3")
                                s4 = epool.tile([128, 512], F32, name=f"s4_{piece}_{m}_{g}", tag="s4")
                                rotE = epool.tile([128, 512], F32R, name=f"re_{piece}_{m}_{g}", tag="rotE")
                                rotO = epool.tile([128, 512], F32R, name=f"ro_{piece}_{m}_{g}", tag="rotO")
                                nc.vector.tensor_mul(s1[:], psA, csl)
                                nc.vector.tensor_mul(s2[:], psB, ssl)
                                nc.vector.tensor_sub(rotE[:], s1[:], s2[:])
                                nc.vector.tensor_mul(s3[:], psA, ssl)
                                nc.vector.tensor_mul(s4[:], psB, csl)
                                nc.vector.tensor_add(rotO[:], s3[:], s4[:])
                                base = piece * F + 256 * g
                                nc.sync.dma_start(
                                    qkt_d[base : base + 128, 512 * m : 512 * (m + 1)],
                                    rotE[:],
                                )
                                nc.sync.dma_start(
                                    qkt_d[base + 128 : base + 256, 512 * m : 512 * (m + 1)],
                                    rotO[:],
                                )
                        else:
                            for f in range(4):
                                vsb = epool.tile([128, F], F32R, name=f"vsb_{m}_{f}", tag="vsb")
                                nc.vector.tensor_copy(vsb[:], ps[f][:])
                                nc.sync.dma_start(
                                    v_d[512 * m + 128 * f : 512 * m + 128 * (f + 1), :],
                                    vsb[:],
                                )
                    # adapter matmuls while this piece's weights are resident
                    if piece == 1:
                        # adapter k: per head, gather its even (chunk 2g) and
                        # odd (chunk 2g+1) dims into one [128, AT] column
                        # block so akt_sb[:, AT*h:+AT] is a ready lhsT.
                        with tc.tile_pool(name="apk", bufs=1) as apool:
                            akpsE = pspool.tile([64, HPC * AT], F32, name="akpsE", tag="ps0")
                            akpsO = pspool.tile([64, HPC * AT], F32, name="akpsO", tag="ps1")
                            for cc in range(CCH):
                                ap_t = apool.tile([128, AT], F32R, name=f"apk_{cc}", tag="ap")
                                nc.sync.dma_start(
                                    ap_t[:], apt[128 * cc : 128 * (cc + 1), :].bitcast(F32R)
                                )
                                for h in range(HPC):
                                    g, idx = h // 2, h % 2
                                    ecol = 256 * g + 64 * idx
                                    ocol = 256 * g + 128 + 64 * idx
                                    nc.tensor.matmul(
                                        akpsE[:, AT * h : AT * (h + 1)],
                                        w_ts[cc][:, ecol : ecol + 64],
                                        ap_t[:],
                                        start=(cc == 0),
                                        stop=(cc == CCH - 1),
                                    )
                                    nc.tensor.matmul(
                                        akpsO[:, AT * h : AT * (h + 1)],
                                        w_ts[cc][:, ocol : ocol + 64],
                                        ap_t[:],
                                        start=(cc == 0),
                                        stop=(cc == CCH - 1),
                                    )
                            tmpE = apool.tile([64, HPC * AT], F32R, name="tmpE")
                            tmpO = apool.tile([64, HPC * AT], F32R, name="tmpO")
                            nc.vector.tensor_copy(tmpE[:], akpsE[:])
                            nc.vector.tensor_copy(tmpO[:], akpsO[:])
                            nc.sync.dma_start(akt_sb[0:64, :], tmpE[:])
                            nc.sync.dma_start(akt_sb[64:128, :], tmpO[:])
                    if piece == 2:
                        with tc.tile_pool(name="apv", bufs=1) as apool:
                            avps = pspool.tile([AT, F], F32, name="avps", tag="ps0")
                            for cc in range(CCH):
                                ap_t = apool.tile([128, AT], F32R, name=f"apv_{cc}", tag="ap")
                                nc.sync.dma_start(
                                    ap_t[:], apt[128 * cc : 128 * (cc + 1), :].bitcast(F32R)
                                )
                                nc.tensor.matmul(
                                    avps[:],
                                    ap_t[:],
                                    w_ts[cc][:],
                                    start=(cc == 0),
                                    stop=(cc == CCH - 1),
                                )
                            nc.vector.tensor_copy(av_sb[:], avps[:])

            # ================= Phase 2: attention =================
            with tc.tile_pool(name="qk", bufs=2) as qkpool, tc.tile_pool(
                name="vt", bufs=2
            ) as vpool, tc.tile_pool(name="pp", bufs=3) as ppool, tc.tile_pool(
                name="cmb", bufs=2
            ) as cpool2, tc.tile_pool(
                name="sps", bufs=2, space="PSUM"
            ) as spspool, tc.tile_pool(
                name="acc", bufs=1, space="PSUM"
            ) as accpool:
                for b in range(B):
                    for hh in range(HPC):
                        bh = b * HPC + hh
                        qT = qkpool.tile([128, T], F32R, name=f"qT_{bh}", tag="qT")
                        kT = qkpool.tile([128, T], F32R, name=f"kT_{bh}", tag="kT")
                        g, idx = hh // 2, hh % 2
                        erow = 256 * g + 64 * idx
                        orow = 256 * g + 128 + 64 * idx
                        for dst, off in ((qT, 0), (kT, F)):
                            nc.sync.dma_start(
                                dst[0:64, :],
                                qkt_d[off + erow : off + erow + 64, b * T : (b + 1) * T],
                            )
                            nc.sync.dma_start(
                                dst[64:128, :],
                                qkt_d[off + orow : off + orow + 64, b * T : (b + 1) * T],
                            )
                        v_ts = []
                        for j in range(T // 128):
                            vt_ = vpool.tile([128, 128], F32R, name=f"v_{bh}_{j}", tag=f"v{j}")
                            nc.sync.dma_start(
                                vt_[:],
                                v_d[
                                    b * T + 128 * j : b * T + 128 * (j + 1),
                                    128 * hh : 128 * (hh + 1),
                                ],
                            )
                            v_ts.append(vt_)
                        for i in range(T // 512):
                            nch = 4 * (i + 1)  # causal Tk chunks for this Tq super
                            y_ps = accpool.tile([128, 512], F32, name=f"y_{bh}_{i}", tag="y")
                            d_ps = accpool.tile([128, 512], F32, name=f"d_{bh}_{i}", tag="d")
                            qsl = qT[:, 512 * i : 512 * (i + 1)]
                            for jj in range(nch // 2):
                                j0, j1 = 2 * jj, 2 * jj + 1
                                s_ps = spspool.tile(
                                    [128, 1024], F32, name=f"s_{bh}_{i}_{jj}", tag="s"
                                )
                                nc.tensor.matmul(
                                    s_ps[:, 0:512],
                                    kT[:, 128 * j0 : 128 * (j0 + 1)],
                                    qsl,
                                    start=True,
                                    stop=True,
                                )
                                nc.tensor.matmul(
                                    s_ps[:, 512:1024],
                                    kT[:, 128 * j1 : 128 * (j1 + 1)],
                                    qsl,
                                    start=True,
                                    stop=True,
                                )
                                p_sb = ppool.tile(
                                    [128, 1024], F32R, name=f"p_{bh}_{i}_{jj}", tag="p"
                                )
                                nc.scalar.activation(p_sb[:], s_ps[:], EXP, scale=SCALE)
                                for j in (j0, j1):
                                    r = j - 4 * i
                                    if r >= 0:
                                        sl = p_sb[:, (j - j0) * 512 : (j - j0) * 512 + 512]
                                        nc.vector.tensor_mul(sl, sl, mask_ts[r][:])
                                for j in (j0, j1):
                                    psl = p_sb[:, (j - j0) * 512 : (j - j0) * 512 + 512]
                                    nc.tensor.matmul(
                                        y_ps[:],
                                        v_ts[j][:],
                                        psl,
                                        start=(j == 0),
                                        stop=(j == nch - 1),
                                    )
                                    nc.tensor.matmul(
                                        d_ps[:],
                                        ones_t[:],
                                        psl,
                                        start=(j == 0),
                                        stop=(j == nch - 1),
                                    )
                            # adapter attention for this Tq super
                            as_ps = spspool.tile([AT, 512], F32, name=f"as_{bh}_{i}", tag="s")
                            nc.tensor.matmul(
                                as_ps[:],
                                akt_sb[:, AT * hh : AT * (hh + 1)],
                                qsl,
                                start=True,
                                stop=True,
                            )
                            ap_sb = ppool.tile([AT, 512], F32R, name=f"ap_{bh}_{i}", tag="ap2")
                            nc.scalar.activation(ap_sb[:], as_ps[:], EXP, scale=ASCALE)
                            ya_ps = accpool.tile([128, 512], F32, name=f"ya_{bh}_{i}", tag="ya")
                            da_ps = accpool.tile([128, 512], F32, name=f"da_{bh}_{i}", tag="da")
                            nc.tensor.matmul(
                                ya_ps[:],
                                av_sb[:, 128 * hh : 128 * (hh + 1)],
                                ap_sb[:],
                                start=True,
                                stop=True,
                            )
                            nc.tensor.matmul(
                                da_ps[:], ones_t[0:AT, :], ap_sb[:], start=True, stop=True
                            )
                            # combine: y/denom + g * ya/denom_a
                            rm = cpool2.tile([128, 512], F32, name=f"rm_{bh}_{i}", tag="rm")
                            ra = cpool2.tile([128, 512], F32, name=f"ra_{bh}_{i}", tag="ra")
                            nc.vector.reciprocal(rm[:], d_ps[:])
                            nc.vector.reciprocal(ra[:], da_ps[:])
                            yn = cpool2.tile([128, 512], F32, name=f"yn_{bh}_{i}", tag="yn")
                            tmp = cpool2.tile([128, 512], F32, name=f"tmp_{bh}_{i}", tag="tmp")
                            yout = cpool2.tile([128, 512], F32, name=f"yo_{bh}_{i}", tag="yo")
                            nc.vector.tensor_mul(yn[:], y_ps[:], rm[:])
                            nc.vector.tensor_mul(tmp[:], ya_ps[:], ra[:])
                            nc.vector.scalar_tensor_tensor(
                                yout[:],
                                tmp[:],
                                gb_t[:, 0:1],
                                yn[:],
                                mybir.AluOpType.mult,
                                mybir.AluOpType.add,
                            )
                            nc.sync.dma_start(
                                yt_in[b][
                                    128 * hh : 128 * (hh + 1),
                                    512 * i : 512 * (i + 1),
                                ],
                                yout[:],
                            )

                    # AllGather this batch's y^T as soon as its heads finish,
                    # overlapping the next batch's attention / projection.
                    nc.gpsimd.collective_compute(
                        "AllGather",
                        mybir.AluOpType.bypass,
                        replica_groups=[list(range(NC))],
                        ins=[yt_in[b][:]],
                        outs=[yt_all[b][:]],
                    )

            # ================= Phase 3: output projection =================
            with tc.tile_pool(name="wpp", bufs=1) as wpool, tc.tile_pool(
                name="yts", bufs=6
            ) as ypool, tc.tile_pool(name="oev", bufs=4) as opool, tc.tile_pool(
                name="ops", bufs=2, space="PSUM"
            ) as pspool:
                wp_ts = []
                for cc in range(CCH):
                    wt = wpool.tile([128, F], F32R, name=f"wp_{cc}")
                    nc.sync.dma_start(
                        wt[:], wp[128 * cc : 128 * (cc + 1), :].bitcast(F32R)
                    )
                    wp_ts.append(wt)
                for s in range(NT // 512):
                    ps = [
                        pspool.tile([128, 512], F32, name=f"o_{s}_{tb}", tag=f"o{tb}")
                        for tb in range(4)
                    ]
                    sb, sl = s // (T // 512), s % (T // 512)
                    for cc in range(CCH):
                        yt_t = ypool.tile([128, 512], F32R, name=f"yt_{s}_{cc}", tag="yt")
                        nc.sync.dma_start(
                            yt_t[:],
                            yt_all[sb][
                                128 * cc : 128 * (cc + 1), 512 * sl : 512 * (sl + 1)
                            ].bitcast(F32R),
                        )
                        for tb in range(4):
                            nc.tensor.matmul(
                                ps[tb][:],
                                yt_t[:, 128 * tb : 128 * (tb + 1)],
                                wp_ts[cc][:],
                                start=(cc == 0),
                                stop=(cc == CCH - 1),
                            )
                    for tb in range(4):
                        osb = opool.tile([128, F], F32, name=f"ob_{s}_{tb}", tag="ob")
                        nc.vector.tensor_copy(osb[:], ps[tb][:])
                        nc.sync.dma_start(
                            out[512 * s + 128 * tb : 512 * s + 128 * (tb + 1), :],
                            osb[:],
                        )
    nc.compile()
    _NC_CACHE["nc"] = nc
    return nc


def _prep_inputs(x, adaption_prompt, rope_cache, w_attn, w_proj, gating_factor):
    x = np.asarray(x, dtype=np.float32)
    w_attn = np.asarray(w_attn, dtype=np.float32)
    w_proj = np.asarray(w_proj, dtype=np.float32)
    rope_cache = np.asarray(rope_cache, dtype=np.float32)
    adaption_prompt = np.asarray(adaption_prompt, dtype=np.float32)
    g = float(np.asarray(gating_factor).reshape(-1)[0])

    xt = np.ascontiguousarray(x.reshape(NT, C).T)
    ang = rope_cache[:T]  # [T, 64]
    cos1 = np.cos(ang).T.astype(np.float32)  # [64, T]
    sin1 = np.sin(ang).T.astype(np.float32)
    cost = np.ascontiguousarray(np.concatenate([cos1, cos1], axis=0))
    sint = np.ascontiguousarray(np.concatenate([sin1, sin1], axis=0))
    ev = np.arange(0, HS, 2)
    od = np.arange(1, HS, 2)
    masks = np.zeros((512, 512), dtype=np.float32)
    tk = np.arange(128)[:, None]
    tq = np.arange(512)[None, :]
    for r in range(4):
        masks[128 * r : 128 * (r + 1)] = (128 * r + tk <= tq).astype(np.float32)
    onesd = np.ones((128, 128), dtype=np.float32)
    gbd = np.full((128, 1), g, dtype=np.float32)
    apt = np.ascontiguousarray(adaption_prompt[0].T)  # [C, AT]

    in_maps = []
    for c in range(NC):
        heads = list(range(HPC * c, HPC * (c + 1)))
        # chunk layout: [h0e|h1e, h0o|h1o, h2e|h3e, h2o|h3o]
        qcols = np.concatenate(
            [
                np.concatenate([heads[2 * g + 0] * HS + par, heads[2 * g + 1] * HS + par])
                for g in range(HPC // 2)
                for par in (ev, od)
            ]
        )
        kcols = C + qcols
        vcols = np.concatenate([2 * C + h * HS + np.arange(HS) for h in heads])
        wqk = np.ascontiguousarray(w_attn[:, np.concatenate([qcols, kcols])])
        wv = np.ascontiguousarray(w_attn[:, vcols])
        wpc = np.ascontiguousarray(w_proj[:, F * c : F * (c + 1)])
        in_maps.append(
            {
                "xt": xt,
                "wqk": wqk,
                "wv": wv,
                "wp": wpc,
                "cost": cost,
                "sint": sint,
                "masks": masks,
                "onesd": onesd,
                "gbd": gbd,
                "apt": apt,
            }
        )
    return in_maps


def kernel(x, adaption_prompt, rope_cache, w_attn, w_proj, gating_factor):
    nc = _build()
    in_maps = _prep_inputs(
        x, adaption_prompt, rope_cache, w_attn, w_proj, gating_factor
    )
    res = run_bass_kernel_spmd(nc, in_maps, core_ids=list(range(NC)))
    full = np.concatenate([res.results[c]["out"] for c in range(NC)], axis=1)
    return full.reshape(B, T, C)
